# revision 23
# baseline (speedup 1.0000x reference)
"""Trainium2 Bass kernel for nn_AttentiveBP (min-plus BP + belief + loss).

The network's outputs (loss, cost_mean) depend only on the min-plus factor
updates, the belief scatter-sum, the softmax/entropy, and the bilinear cost
terms; the GAT/GRU/attention subgraph writes msgs[0:2F] while belief reads
msgs[2F:4F], so it is dead w.r.t. the outputs and skipped.

Three SPMD NEFFs over 8 cores with host-side index shuffling only:
  K1: stream cost_tensors, convert to fp16 on Act, min-plus via fp16
      tensor_tensor min-trees (DVE 2x mode) in a g-innermost layout;
      broadcast adds split DVE/Pool. Small leading tiles cut pipeline fill.
  K2: belief = tightly-packed per-window segment sums (degree-sorted v
      permutation, per-window slot depth); argmin + softmax in fp32;
      entropy via sum(dist*ln dist) = -sum(dist*bel) - ln(den).
  K3: stream cost_tensors again; fp16 outer-product (split DVE/Pool) +
      scalar_tensor_tensor accumulation for the bilinear term.
"""
import os
import sys

sys.path.insert(0, "/opt/trn_rl_repo")

import numpy as np

import concourse.bass as bass
import concourse.bacc as bacc
import concourse.tile as tile
from concourse import mybir
from concourse.bass_utils import run_bass_kernel_spmd

F_N = 100000
V_N = 30000
D = 15
NCORES = 8
FPC = F_N // NCORES          # 12500 factors per core
P = 128
NCH = (FPC + P - 1) // P     # 98 chunks of 128 factors
FPAD = NCH * P               # 12544 padded factors per core
# variable tile sizes: small leading tiles shorten the pipeline-fill stall,
# small trailing tiles shorten the drain chain
TILES = ([(0, 2), (2, 4)] + [(6 + 8 * k, 8) for k in range(11)]
         + [(94, 2), (96, 2)])
assert sum(g for _, g in TILES) == NCH
FTOT = NCH * D               # per-partition elements of m / msg buffers
VPC = V_N // NCORES          # 3750 v per core
NW = (VPC + P - 1) // P      # 30 windows
VPAD = NW * P                # 3840

FP32 = mybir.dt.float32
FP16 = mybir.dt.float16
I32 = mybir.dt.int32
AX = mybir.AxisListType
OP = mybir.AluOpType
ACT = mybir.ActivationFunctionType

last_exec_times = []

_cache = {}


def _tile_off(ti):
    return sum(D * g for _, g in TILES[:ti])


def _build_k1():
    nc = bacc.Bacc(None)
    c_in = nc.dram_tensor("c_in", [FPAD, D * D], FP32, kind="ExternalInput")
    # [P, FTOT] per-tile [D, g] blocks (g innermost)
    mrv_in = nc.dram_tensor("mrv_in", [P, FTOT], FP32, kind="ExternalInput")
    mcv_in = nc.dram_tensor("mcv_in", [P, FTOT], FP32, kind="ExternalInput")
    m1_out = nc.dram_tensor("m1_out", [P, FTOT], FP16, kind="ExternalOutput")
    m2_out = nc.dram_tensor("m2_out", [P, FTOT], FP16, kind="ExternalOutput")

    NT = len(TILES)
    # msg chunks (tile-index ranges): staggered loads so tile-0 starts early
    CH = [(0, 2), (2, 4), (6, 4), (10, NT - 10)]
    SP = 14  # i rows of s2 handled by Pool (Add eff 0.42)
    HALF = 7  # emit first m-output DMA after this many tiles

    with tile.TileContext(nc) as tc:
        with tc.tile_pool(name="cts", bufs=4) as cpool, \
             tc.tile_pool(name="c16p", bufs=3) as c16p, \
             tc.tile_pool(name="scr", bufs=3) as spool, \
             tc.tile_pool(name="tr", bufs=3) as trp, \
             tc.tile_pool(name="sb", bufs=1) as sb:
            m16 = {}   # ("mcv"/"mrv", tile) -> (chunk tile, elem offset)
            mst1 = sb.tile([P, FTOT], FP16)
            mst2 = sb.tile([P, FTOT], FP16)
            for ti, (g0, g) in enumerate(TILES):
                off = _tile_off(ti)
                ct = cpool.tile([P, G8 := 8, D * D], FP32, tag="ct")
                src = bass.AP(tensor=c_in[:].tensor, offset=g0 * P * D * D,
                              ap=[[D * D, P], [P * D * D, g], [1, D * D]])
                nc.sync.dma_start(out=ct[:, :g, :], in_=src)

                # Act: convert to fp16 in [i, j, g] layout (g innermost)
                c16 = c16p.tile([P, D, D, G8], FP16, tag="c16")
                cin_ap = bass.AP(tensor=ct.tensor, offset=ct.offset,
                                 ap=[ct.ap[0], [D * D, g], [D, D], [1, D]])
                cout_ap = bass.AP(tensor=c16.tensor, offset=c16.offset,
                                  ap=[c16.ap[0], [1, g], [D * G8, D], [G8, D]])
                nc.scalar.activation(out=cout_ap, in_=cin_ap, func=ACT.Copy)

                for ci, (t0, nt) in enumerate(CH):
                    if t0 != ti:
                        continue
                    o0 = _tile_off(t0)
                    sz = _tile_off(t0 + nt) - o0
                    for nm, src2 in (("mcv", mcv_in), ("mrv", mrv_in)):
                        stg = sb.tile([P, sz], FP32, tag=f"{nm}f{t0}")
                        nc.sync.dma_start(out=stg[:], in_=src2[:, o0:o0 + sz])
                        cv = sb.tile([P, sz], FP16, tag=f"{nm}16_{t0}")
                        # early chunks convert on Pool (idle during fill);
                        # later ones on Act
                        eng = nc.gpsimd if ci < 2 else nc.scalar
                        if ci < 2:
                            eng.tensor_copy(out=cv[:], in_=stg[:])
                        else:
                            eng.activation(out=cv[:], in_=stg[:], func=ACT.Copy)
                        for dt in range(nt):
                            m16[(nm, t0 + dt)] = (cv, _tile_off(t0 + dt) - o0)

                # s1[i,j,g] = c16 + mcv[j,g] (bcast over i); DVE 2x
                s1 = spool.tile([P, D, D, G8], FP16, tag="s1")
                mcv_t, mcv_o = m16[("mcv", ti)]
                mcv_b = bass.AP(tensor=mcv_t.tensor, offset=mcv_t.offset + mcv_o,
                                ap=[mcv_t.ap[0], [0, D], [g, D], [1, g]])
                nc.vector.tensor_tensor(out=s1[:, :, :, :g], in0=c16[:, :, :, :g],
                                        in1=mcv_b, op=OP.add)
                # tree-min over j (DVE 2x) -> m1[i, g]
                t1 = trp.tile([P, D, 8, G8], FP16, tag="t1")
                nc.vector.tensor_tensor(out=t1[:, :, :, :g], in0=s1[:, :, 0:8, :g],
                                        in1=s1[:, :, 7:15, :g], op=OP.min)
                t2 = trp.tile([P, D, 4, G8], FP16, tag="t2")
                nc.vector.tensor_tensor(out=t2[:, :, :, :g], in0=t1[:, :, 0:4, :g],
                                        in1=t1[:, :, 4:8, :g], op=OP.min)
                t3 = trp.tile([P, D, 2, G8], FP16, tag="t3")
                nc.vector.tensor_tensor(out=t3[:, :, :, :g], in0=t2[:, :, 0:2, :g],
                                        in1=t2[:, :, 2:4, :g], op=OP.min)
                m1o = bass.AP(tensor=mst1.tensor, offset=mst1.offset + off,
                              ap=[mst1.ap[0], [g, D], [g * D, 1], [1, g]])
                nc.vector.tensor_tensor(out=m1o, in0=t3[:, :, 0:1, :g],
                                        in1=t3[:, :, 1:2, :g], op=OP.min)

                # s2[i,j,g] = c16 + mrv[i,g] (bcast over j); split Pool/DVE
                s2 = spool.tile([P, D, D, G8], FP16, tag="s2")
                mrv_t, mrv_o = m16[("mrv", ti)]
                mrv_lo = bass.AP(tensor=mrv_t.tensor, offset=mrv_t.offset + mrv_o,
                                 ap=[mrv_t.ap[0], [g, SP], [0, D], [1, g]])
                nc.gpsimd.tensor_tensor(out=s2[:, 0:SP, :, :g],
                                        in0=c16[:, 0:SP, :, :g],
                                        in1=mrv_lo, op=OP.add)
                mrv_hi = bass.AP(tensor=mrv_t.tensor,
                                 offset=mrv_t.offset + mrv_o + SP * g,
                                 ap=[mrv_t.ap[0], [g, D - SP], [0, D], [1, g]])
                nc.vector.tensor_tensor(out=s2[:, SP:D, :, :g],
                                        in0=c16[:, SP:D, :, :g],
                                        in1=mrv_hi, op=OP.add)
                # tree-min over i (DVE 2x) -> m2[j, g]
                u1 = trp.tile([P, 8, D, G8], FP16, tag="u1")
                nc.vector.tensor_tensor(out=u1[:, :, :, :g], in0=s2[:, 0:8, :, :g],
                                        in1=s2[:, 7:15, :, :g], op=OP.min)
                u2 = trp.tile([P, 4, D, G8], FP16, tag="u2")
                nc.vector.tensor_tensor(out=u2[:, :, :, :g], in0=u1[:, 0:4, :, :g],
                                        in1=u1[:, 4:8, :, :g], op=OP.min)
                u3 = trp.tile([P, 2, D, G8], FP16, tag="u3")
                nc.vector.tensor_tensor(out=u3[:, :, :, :g], in0=u2[:, 0:2, :, :g],
                                        in1=u2[:, 2:4, :, :g], op=OP.min)
                m2o = bass.AP(tensor=mst2.tensor, offset=mst2.offset + off,
                              ap=[mst2.ap[0], [g * D, 1], [g, D], [1, g]])
                nc.vector.tensor_tensor(out=m2o, in0=u3[:, 0:1, :, :g],
                                        in1=u3[:, 1:2, :, :g], op=OP.min)

                if ti == HALF - 1:
                    h = _tile_off(HALF)
                    nc.sync.dma_start(out=m1_out[:, 0:h], in_=mst1[:, 0:h])
                    nc.sync.dma_start(out=m2_out[:, 0:h], in_=mst2[:, 0:h])
                elif ti == NT - 2:
                    h, h2 = _tile_off(HALF), _tile_off(NT - 1)
                    nc.sync.dma_start(out=m1_out[:, h:h2], in_=mst1[:, h:h2])
                    nc.sync.dma_start(out=m2_out[:, h:h2], in_=mst2[:, h:h2])
                elif ti == NT - 1:
                    h2 = _tile_off(NT - 1)
                    nc.sync.dma_start(out=m1_out[:, h2:FTOT], in_=mst1[:, h2:FTOT])
                    nc.sync.dma_start(out=m2_out[:, h2:FTOT], in_=mst2[:, h2:FTOT])
    nc.compile()
    return nc


def _build_k2(kws):
    """kws: tuple of per-window slot depths (same on all cores)."""
    FT = sum(k * D for k in kws)
    offs = []
    o = 0
    for k in kws:
        offs.append(o)
        o += k * D

    nc = bacc.Bacc(None)
    slots_in = nc.dram_tensor("slots_in", [P, FT], FP16, kind="ExternalInput")
    vmask_in = nc.dram_tensor("vmask_in", [P, NW], FP32, kind="ExternalInput")
    iotad_in = nc.dram_tensor("iotad_in", [P, D], FP32, kind="ExternalInput")
    table_out = nc.dram_tensor("table_out", [P, NW, 16], FP16, kind="ExternalOutput")
    ent_out = nc.dram_tensor("ent_out", [P, 1], FP32, kind="ExternalOutput")

    # group equal-depth window runs
    runs = []
    w = 0
    while w < NW:
        w2 = w
        while w2 < NW and kws[w2] == kws[w]:
            w2 += 1
        runs.append((w, w2, kws[w]))
        w = w2
    # pack runs into DMA groups (first small so the first reduce starts early)
    groups = []
    cur = []
    acc = 0
    lim = FT // 6
    for r in runs:
        cur.append(r)
        acc += (r[1] - r[0]) * r[2] * D
        if acc > lim:
            groups.append(cur)
            cur = []
            acc = 0
            lim = FT // 3
    if cur:
        groups.append(cur)

    with tile.TileContext(nc) as tc:
        with tc.tile_pool(name="sb", bufs=1) as sb:
            # belief[p, w, d] = sum_k slots[p, off_w + k*D + d]
            sl = sb.tile([P, FT], FP16)
            bel = sb.tile([P, NW, D], FP32)
            vmask = sb.tile([P, NW], FP32)
            iotad = sb.tile([P, D], FP32)
            for gi, grp in enumerate(groups):
                o0 = offs[grp[0][0]]
                o1 = offs[grp[-1][1] - 1] + grp[-1][2] * D
                nc.sync.dma_start(out=sl[:, o0:o1], in_=slots_in[:, o0:o1])
                if gi == 0:
                    nc.sync.dma_start(out=vmask[:], in_=vmask_in[:])
                    nc.sync.dma_start(out=iotad[:], in_=iotad_in[:])
                for w, w2, k in grp:
                    src = bass.AP(tensor=sl.tensor, offset=sl.offset + offs[w],
                                  ap=[sl.ap[0], [k * D, w2 - w], [1, D], [D, k]])
                    nc.vector.tensor_reduce(out=bel[:, w:w2, :], in_=src,
                                            axis=AX.X, op=OP.add)

            # argmax(dist) == argmin(bel), first-index tie-break via +iota*eps
            iota_b = bass.AP(tensor=iotad.tensor, offset=iotad.offset,
                             ap=[iotad.ap[0], [0, NW], [1, D]])
            dtb = sb.tile([P, NW, D], FP32)
            nc.vector.scalar_tensor_tensor(out=dtb[:], in0=iota_b, scalar=1e-5,
                                           in1=bel[:], op0=OP.mult, op1=OP.add)
            mn = sb.tile([P, NW], FP32)
            nc.vector.tensor_reduce(out=mn[:], in_=dtb[:], axis=AX.X, op=OP.min)
            ohm = sb.tile([P, NW, D], FP32)
            mn_b = bass.AP(tensor=mn.tensor, offset=mn.offset,
                           ap=[mn.ap[0], mn.ap[1], [0, D]])
            nc.vector.tensor_tensor(out=ohm[:], in0=dtb[:], in1=mn_b, op=OP.is_equal)
            amax = sb.tile([P, NW], FP32)
            tmp = sb.tile([P, NW, D], FP32)
            nc.vector.tensor_tensor(out=tmp[:], in0=ohm[:], in1=iota_b, op=OP.mult)
            nc.vector.tensor_reduce(out=amax[:], in_=tmp[:], axis=AX.X, op=OP.add)
            tbl = sb.tile([P, NW, 16], FP16)
            amax3 = bass.AP(tensor=amax.tensor, offset=amax.offset,
                            ap=[amax.ap[0], amax.ap[1], [1, 1]])
            nc.vector.tensor_copy(out=tbl[:, :, D:D + 1], in_=amax3)

            # dist = exp(-bel) / sum_d  (fp32; |bel| small); dist lands
            # directly in the fp16 table
            e = sb.tile([P, NW, D], FP32)
            nc.scalar.activation(out=e[:], in_=bel[:], func=ACT.Exp, scale=-1.0)
            den = sb.tile([P, NW], FP32)
            nc.vector.tensor_reduce(out=den[:], in_=e[:], axis=AX.X, op=OP.add)
            rden = sb.tile([P, NW], FP32)
            nc.vector.reciprocal(out=rden[:], in_=den[:])
            rden_b = bass.AP(tensor=rden.tensor, offset=rden.offset,
                             ap=[rden.ap[0], rden.ap[1], [0, D]])
            nc.vector.tensor_tensor(out=tbl[:, :, 0:D], in0=e[:], in1=rden_b,
                                    op=OP.mult)
            nc.sync.dma_start(out=table_out[:], in_=tbl[:])

            # entropy: sum_d dist*ln(dist) = -(sum_d dist*bel + ln den);
            # host flips the sign (the +1e-6 inside ln is negligible)
            eb = sb.tile([P, NW, D], FP32)
            nc.gpsimd.tensor_tensor(out=eb[:], in0=e[:], in1=bel[:], op=OP.mult)
            ebs = sb.tile([P, NW], FP32)
            nc.vector.tensor_reduce(out=ebs[:], in_=eb[:], axis=AX.X, op=OP.add)
            lnden = sb.tile([P, NW], FP32)
            nc.scalar.activation(out=lnden[:], in_=den[:], func=ACT.Ln)
            t1 = sb.tile([P, NW], FP32)
            nc.vector.tensor_tensor(out=t1[:], in0=ebs[:], in1=rden[:], op=OP.mult)
            t2 = sb.tile([P, NW], FP32)
            nc.vector.tensor_tensor(out=t2[:], in0=t1[:], in1=lnden[:], op=OP.add)
            entp = sb.tile([P, 1], FP32)
            dead = sb.tile([P, NW], FP32)
            nc.vector.scalar_tensor_tensor(out=dead[:], in0=t2[:], scalar=1.0,
                                           in1=vmask[:], op0=OP.mult, op1=OP.mult,
                                           accum_out=entp[:])
            nc.sync.dma_start(out=ent_out[:], in_=entp[:])
    nc.compile()
    return nc


def _build_k3():
    nc = bacc.Bacc(None)
    c_in = nc.dram_tensor("c_in", [FPAD * D * D], FP32, kind="ExternalInput")
    drv_in = nc.dram_tensor("drv_in", [P, FTOT], FP16, kind="ExternalInput")
    dcv_in = nc.dram_tensor("dcv_in", [P, FTOT], FP16, kind="ExternalInput")
    cval_in = nc.dram_tensor("cval_in", [P, NCH], FP32, kind="ExternalInput")
    per_out = nc.dram_tensor("per_out", [P, 1], FP32, kind="ExternalOutput")
    cost_out = nc.dram_tensor("cost_out", [P, 1], FP32, kind="ExternalOutput")

    NT = len(TILES)

    with tile.TileContext(nc) as tc:
        with tc.tile_pool(name="cts", bufs=4) as cpool, \
             tc.tile_pool(name="c16p", bufs=3) as c16p, \
             tc.tile_pool(name="scr", bufs=3) as spool, \
             tc.tile_pool(name="sb", bufs=1) as sb:
            perC = sb.tile([P, NT], FP32)
            rows = {}
            for ti, (g0, g) in enumerate(TILES):
                off = _tile_off(ti)
                ct = cpool.tile([P, 8, D * D], FP32, tag="ct")
                src = bass.AP(tensor=c_in[:].tensor, offset=g0 * P * D * D,
                              ap=[[D * D, P], [P * D * D, g], [1, D * D]])
                nc.sync.dma_start(out=ct[:, :g, :], in_=src)

                c16 = c16p.tile([P, D, D, 8], FP16, tag="c16")
                cin_ap = bass.AP(tensor=ct.tensor, offset=ct.offset,
                                 ap=[ct.ap[0], [D * D, g], [D, D], [1, D]])
                cout_ap = bass.AP(tensor=c16.tensor, offset=c16.offset,
                                  ap=[c16.ap[0], [1, g], [D * 8, D], [8, D]])
                nc.scalar.activation(out=cout_ap, in_=cin_ap, func=ACT.Copy)

                if ti == 0:
                    drv = sb.tile([P, FTOT], FP16)
                    nc.sync.dma_start(out=drv[:], in_=drv_in[:])
                    dcv = sb.tile([P, FTOT], FP16)
                    nc.sync.dma_start(out=dcv[:], in_=dcv_in[:])
                    rows["drv"], rows["dcv"] = drv, dcv
                elif ti == 2:
                    cvals = sb.tile([P, NCH], FP32)
                    nc.sync.dma_start(out=cvals[:], in_=cval_in[:])
                    costp = sb.tile([P, 1], FP32)
                    nc.vector.tensor_reduce(out=costp[:], in_=cvals[:],
                                            axis=AX.X, op=OP.add)
                    nc.sync.dma_start(out=cost_out[:], in_=costp[:])

                drv, dcv = rows["drv"], rows["dcv"]
                # U[i,j,g] = drv[i,g] * dcv[j,g]; split Pool (10/15) / DVE
                SU = 10
                u = spool.tile([P, D, D, 8], FP16, tag="u")
                drv_lo = bass.AP(tensor=drv.tensor, offset=drv.offset + off,
                                 ap=[drv.ap[0], [g, SU], [0, D], [1, g]])
                dcv_b1 = bass.AP(tensor=dcv.tensor, offset=dcv.offset + off,
                                 ap=[dcv.ap[0], [0, SU], [g, D], [1, g]])
                nc.gpsimd.tensor_tensor(out=u[:, 0:SU, :, :g], in0=drv_lo,
                                        in1=dcv_b1, op=OP.mult)
                drv_hi = bass.AP(tensor=drv.tensor,
                                 offset=drv.offset + off + SU * g,
                                 ap=[drv.ap[0], [g, D - SU], [0, D], [1, g]])
                dcv_b2 = bass.AP(tensor=dcv.tensor, offset=dcv.offset + off,
                                 ap=[dcv.ap[0], [0, D - SU], [g, D], [1, g]])
                nc.vector.tensor_tensor(out=u[:, SU:D, :, :g], in0=drv_hi,
                                        in1=dcv_b2, op=OP.mult)
                # per += sum c16 * U; STT (1x, DVE-only op)
                dead = spool.tile([P, D, D, 8], FP16, tag="dead")
                nc.vector.scalar_tensor_tensor(out=dead[:, :, :, :g],
                                               in0=c16[:, :, :, :g], scalar=1.0,
                                               in1=u[:, :, :, :g], op0=OP.mult,
                                               op1=OP.mult,
                                               accum_out=perC[:, ti:ti + 1])
            perp = sb.tile([P, 1], FP32)
            nc.vector.tensor_reduce(out=perp[:], in_=perC[:], axis=AX.X, op=OP.add)
            nc.sync.dma_start(out=per_out[:], in_=perp[:])
    nc.compile()
    return nc


def _get_k1():
    if "k1" not in _cache:
        _cache["k1"] = _build_k1()
    return _cache["k1"]


def _get_k2(kws):
    key = ("k2", kws)
    if key not in _cache:
        _cache[key] = _build_k2(kws)
    return _cache[key]


def _get_k3():
    if "k3" not in _cache:
        _cache["k3"] = _build_k3()
    return _cache["k3"]


def _to_tiles(rows, dtype):
    """[FPC, D] -> [P, FTOT] with per-tile [D, g] blocks (g innermost)."""
    pad = np.zeros((FPAD, D), dtype)
    pad[:FPC] = rows
    ch = pad.reshape(NCH, P, D)
    out = np.empty((P, FTOT), dtype)
    for ti, (g0, g) in enumerate(TILES):
        off = _tile_off(ti)
        blk = ch[g0:g0 + g]                     # [g, P, D]
        out[:, off:off + D * g] = (
            blk.transpose(1, 2, 0).reshape(P, D * g))
    return np.ascontiguousarray(out)


def _from_tiles(arr):
    """[P, FTOT] -> [FPC, D]."""
    ch = np.empty((NCH, P, D), arr.dtype)
    for ti, (g0, g) in enumerate(TILES):
        off = _tile_off(ti)
        blk = arr[:, off:off + D * g].reshape(P, D, g)
        ch[g0:g0 + g] = blk.transpose(2, 0, 1)
    return ch.reshape(FPAD, D)[:FPC]


def kernel(**inp):
    global last_exec_times
    last_exec_times = []
    f32 = np.float32
    f16 = np.float16

    msgs = np.asarray(inp["msgs"], f32)
    C = np.ascontiguousarray(np.asarray(inp["cost_tensors"], f32).reshape(F_N, D * D))
    rv2f_idx = np.asarray(inp["msg_rv2f_idxes"], np.int64)
    cv2f_idx = np.asarray(inp["msg_cv2f_idxes"], np.int64)
    f2rv_idx = np.asarray(inp["msg_f2rv_idxes"], np.int64)
    f2cv_idx = np.asarray(inp["msg_f2cv_idxes"], np.int64)
    f2v_idx = np.asarray(inp["msg_f2v_per_v_idxes"], np.int64)
    scat = np.asarray(inp["f2v_per_v_scatter_idxes"], np.int64)
    rv_idx = np.asarray(inp["rv_idxes"], np.int64)
    cv_idx = np.asarray(inp["cv_idxes"], np.int64)

    m_rv2f = msgs[rv2f_idx]   # [F, D]
    m_cv2f = msgs[cv2f_idx]

    trace = bool(int(os.environ.get("KERNEL_TRACE", "0")))

    # ---------------- K1: min-plus ----------------
    k1 = _get_k1()
    in_maps1 = []
    cslices = []
    for c in range(NCORES):
        lo, hi = c * FPC, (c + 1) * FPC
        cs = np.zeros((FPAD, D * D), f32)
        cs[:FPC] = C[lo:hi]
        cslices.append(cs)
        in_maps1.append(dict(c_in=cs,
                             mrv_in=_to_tiles(m_rv2f[lo:hi], f32),
                             mcv_in=_to_tiles(m_cv2f[lo:hi], f32)))
    r1 = run_bass_kernel_spmd(k1, in_maps1, core_ids=list(range(NCORES)),
                              trace=trace)
    if r1.exec_time_ns:
        last_exec_times.append(r1.exec_time_ns)

    # m rows in [2F, 4F) index space (f2rv/f2cv are arange per the problem
    # spec, so the min-plus outputs cover every row belief reads)
    m16 = np.zeros((2 * F_N, D), f16)
    for c in range(NCORES):
        lo, hi = c * FPC, (c + 1) * FPC
        m1 = _from_tiles(np.asarray(r1.results[c]["m1_out"]))
        m2 = _from_tiles(np.asarray(r1.results[c]["m2_out"]))
        m16[f2rv_idx[lo:hi] - 2 * F_N] = m1
        m16[f2cv_idx[lo:hi] - 2 * F_N] = m2

    # ---------------- host relay: degree-sorted packed slots ----------------
    counts = np.bincount(scat, minlength=V_N)
    vsort = np.argsort(-counts, kind="stable")   # v by count desc
    vrank = np.empty(V_N, np.int64)
    vrank[vsort] = np.arange(V_N)
    # rank r -> core r%8, slot s=r//8, window s//128, partition s%128
    csort = counts[vsort]
    kws = []
    for w in range(NW):
        blk = csort[w * NCORES * P:(w + 1) * NCORES * P]
        kws.append(max(int(blk.max()) if blk.size else 1, 1))
    kws = tuple(kws)
    offs = np.zeros(NW + 1, np.int64)
    np.cumsum(np.array(kws) * D, out=offs[1:])
    FT = int(offs[-1])
    k2 = _get_k2(kws)

    # entry t: row m16[f2v_idx[t]-2F] added to belief[scat[t]]
    order = np.argsort(scat, kind="stable")
    v_sorted = scat[order]
    startv = np.zeros(V_N + 1, np.int64)
    np.cumsum(counts, out=startv[1:])
    krank = np.arange(2 * F_N) - startv[v_sorted]   # slot within v
    slot_rows = m16[f2v_idx[order] - 2 * F_N]       # [T, D] fp16

    r = vrank[v_sorted]
    core_of = r % NCORES
    s = r // NCORES
    w_of = s // P
    p_of = s % P
    # flat D-row index into slots [NCORES, P, FT]
    row_idx = (core_of * P + p_of) * (FT // D) + offs[w_of] // D + krank
    slots = np.zeros((NCORES * P * (FT // D), D), f16)
    slots[row_idx] = slot_rows
    slots = slots.reshape(NCORES, P, FT)

    vmask = np.zeros((P, NW), f32)
    nact = VPC
    full_w = nact // P
    vmask[:, :full_w] = 1.0
    vmask[:nact - full_w * P, full_w] = 1.0
    iotad = np.broadcast_to(np.arange(D, dtype=f32), (P, D)).copy()

    in_maps2 = [dict(slots_in=slots[c], vmask_in=vmask, iotad_in=iotad)
                for c in range(NCORES)]
    r2 = run_bass_kernel_spmd(k2, in_maps2, core_ids=list(range(NCORES)),
                              trace=trace)
    if r2.exec_time_ns:
        last_exec_times.append(r2.exec_time_ns)

    # table rows addressed by (core, p, w)
    table = np.zeros((NCORES * P * NW, 16), f16)
    ent_nat = 0.0
    for c in range(NCORES):
        tb = np.asarray(r2.results[c]["table_out"])  # [P, NW, 16]
        table[c * P * NW:(c + 1) * P * NW] = tb.reshape(P * NW, 16)
        ent_nat += float(np.asarray(r2.results[c]["ent_out"]).sum())

    rall = vrank
    vrow = (rall % NCORES) * P * NW + (rall // NCORES % P) * NW + rall // (NCORES * P)

    # ---------------- K3: bilinear + cost ----------------
    k3 = _get_k3()
    drv_rows = table[vrow[rv_idx]]  # [F, 16] fp16
    dcv_rows = table[vrow[cv_idx]]
    vr = drv_rows[:, D].astype(np.int64)
    vc = dcv_rows[:, D].astype(np.int64)
    cost_vals = C[np.arange(F_N), vr * D + vc]
    in_maps3 = []
    for c in range(NCORES):
        lo, hi = c * FPC, (c + 1) * FPC
        cvp = np.zeros(FPAD, f32)
        cvp[:FPC] = cost_vals[lo:hi]
        in_maps3.append(dict(
            c_in=cslices[c].reshape(-1),
            drv_in=_to_tiles(drv_rows[lo:hi, :D], f16),
            dcv_in=_to_tiles(dcv_rows[lo:hi, :D], f16),
            cval_in=np.ascontiguousarray(cvp.reshape(NCH, P).T)))
    r3 = run_bass_kernel_spmd(k3, in_maps3, core_ids=list(range(NCORES)),
                              trace=trace)
    if r3.exec_time_ns:
        last_exec_times.append(r3.exec_time_ns)

    per_sum = 0.0
    cost_sum = 0.0
    for c in range(NCORES):
        per_sum += float(np.asarray(r3.results[c]["per_out"]).sum())
        cost_sum += float(np.asarray(r3.results[c]["cost_out"]).sum())

    # entp accumulated sum_w mask*(sum_d dist*bel + ln den) = -sum dist ln dist
    ent = ent_nat / np.log(2.0) / V_N
    loss = per_sum + 0.1 * ent
    cost_mean = cost_sum
    return np.array([loss, cost_mean], dtype=np.float32)
